# revision 8
# baseline (speedup 1.0000x reference)
"""Multi-head causal self-attention (B=4, T=2048, C=768, H=12) on 8 trn2 cores.

Sharding: core c handles batch b = c//2 and head-group hg = c%2 (6 heads each).
Host sums the output-projection partials per batch, transposes back, and adds
b_o. No cross-core collectives.

This revision restructures PV around the cost model's "stationary loads are
free" property: PV runs per (q-block, k-block) 128x128 tile with the att tile
as the stationary operand and vnat (65 cols: 64 v-features + ones) as the
moving operand, cutting PV streaming from 17408 to 8840 columns per head.
The output lands NATURAL [q-token partition, feature], so the softmax
denominator (ones column) sits on the same partition as its token and
normalization is a per-partition DVE reciprocal+multiply -- no Pool
partition-broadcasts at all.  yT for the output projection is rebuilt by
DMA-engine xbar transposes (no PE/PSUM involvement).  Scores/exp/mask and
the QKV/output projections keep the previous structure; output staging
copies are split between DVE and Pool to keep both below the ACT exp load,
which is the end-state bottleneck.
"""

import math
import os
from collections import deque

import numpy as np
import ml_dtypes

import concourse.bass as bass
from concourse import bacc
import concourse.mybir as mybir
import concourse.tile as tile
from concourse import bass_utils
from concourse.bass import ts
from concourse.masks import make_identity

F32 = mybir.dt.float32
BF16 = mybir.dt.bfloat16

P = 128
T = 2048          # sequence length
C = 768           # embed dim
CS = C // P       # 6 contraction chunks
HL = 6            # heads per core
HD = 64           # head dim
J = HL * HD       # 384 local y-feature dim
JS = J // P       # 3
OQK = 2 * J // P  # 6 o-blocks of the local W_qk slice (q rows then k rows)
OUTB = C // P     # 6 output row blocks
TT = T // 512     # 4 column tiles of 512
TB = T // P       # 16 token blocks
VG = HD + 1       # 65: per-head v columns + ones column


def _build_bass():
    nc = bacc.Bacc("TRN2", target_bir_lowering=False, debug=False)
    xt_d = nc.dram_tensor("xt", [C, T], BF16, kind="ExternalInput").ap()
    wqk_d = nc.dram_tensor("wqk", [C, 2 * J], BF16, kind="ExternalInput").ap()
    wv_d = nc.dram_tensor("wv", [C, J], BF16, kind="ExternalInput").ap()
    wo_d = nc.dram_tensor("wo", [J, C], BF16, kind="ExternalInput").ap()
    bqk_d = nc.dram_tensor("bqk", [2 * J], F32, kind="ExternalInput").ap()
    bv_d = nc.dram_tensor("bv", [J], F32, kind="ExternalInput").ap()
    outa_d = nc.dram_tensor("outa", [C, T], BF16, kind="ExternalOutput").ap()
    outb_d = nc.dram_tensor("outb", [C, T], BF16, kind="ExternalOutput").ap()

    with tile.TileContext(nc) as tc, nc.allow_low_precision(
        reason="bf16 matmul pipeline; fp32 PSUM accumulation throughout"
    ):
        _emit_kernel(tc, xt_d, wqk_d, wv_d, wo_d, bqk_d, bv_d, outa_d, outb_d)
    nc.compile()
    return nc


def _emit_kernel(tc, xt_d, wqk_d, wv_d, wo_d, bqk_d, bv_d, outa_d, outb_d):
    nc = tc.nc
    scale = 1.0 / math.sqrt(HD)

    xt_r = xt_d.rearrange("(cb p) t -> p cb t", p=P)     # [128, 6, 2048]
    wqk_r = wqk_d.rearrange("(cb p) o -> p cb o", p=P)   # [128, 6, 768]
    wv_r = wv_d.rearrange("(cb p) j -> p cb j", p=P)     # [128, 6, 384]
    wo_r = wo_d.rearrange("(jb p) o -> p jb o", p=P)     # [128, 3, 768]
    bqk_r = bqk_d.rearrange("(a p) -> p a", p=P)         # [128, 6]
    bv_r = bv_d.rearrange("(p a) -> p a", p=1)           # [1, 384]
    outa_r = outa_d.rearrange("(ob p) t -> p ob t", p=P)  # [128, 6, 2048]
    outb_r = outb_d.rearrange("(ob p) t -> p ob t", p=P)

    with (
        tc.tile_pool(name="persist", bufs=1) as persist,
        tc.tile_pool(name="stage", bufs=2) as stage,
        tc.tile_pool(name="attn", bufs=2) as attn,
        tc.tile_pool(name="ps512", bufs=2, space="PSUM") as ps512,
        tc.tile_pool(name="ps_s", bufs=2, space="PSUM") as ps_s,
        tc.tile_pool(name="ps_y", bufs=1, space="PSUM") as ps_y,
    ):
        xt = persist.tile([P, CS, T], BF16)       # x^T      24KB/partition
        wqk = persist.tile([P, CS, 2 * J], BF16)  # Wqk^T     9KB
        wv = persist.tile([P, CS, J], BF16)       # Wv^T    4.5KB
        wo = persist.tile([P, JS, C], BF16)       # Wo^T    4.5KB
        qkvT = persist.tile([P, OQK, T], BF16)    # [q|k]^T  24KB
        vnat = persist.tile([P, TB, HL * VG], BF16)  # v natural 12.2KB
        yT = persist.tile([P, JS, T], BF16)       # y^T      12KB
        ynat = persist.tile([P, TB, JS, P], BF16)  # y natural 12KB
        bsb = persist.tile([P, OQK], F32)
        bvrow = persist.tile([1, J], F32)
        brep = persist.tile([P, J], F32)

        # ---- input loads. HWDGE issues DMAs serially (~625ns each) and the
        # DMA engines run one transfer at a time (internally 16-way), so use
        # FEW large DMAs (>=512B contiguous runs where possible), ordered so
        # the first compute unit's data lands first.
        nc.sync.dma_start(wqk[:, 0:3, ts(0, P)], wqk_r[:, 0:3, ts(0, P)])
        nc.sync.dma_start(xt[:, 0:3, ts(0, 512)], xt_r[:, 0:3, ts(0, 512)])
        nc.sync.dma_start(wqk[:, 3:6, ts(0, P)], wqk_r[:, 3:6, ts(0, P)])
        nc.sync.dma_start(xt[:, 3:6, ts(0, 512)], xt_r[:, 3:6, ts(0, 512)])
        nc.sync.dma_start(wv, wv_r)
        nc.sync.dma_start(wqk[:, :, ts(3, P)], wqk_r[:, :, ts(3, P)])
        nc.sync.dma_start(bsb, bqk_r)
        nc.sync.dma_start(bvrow, bv_r)
        nc.sync.dma_start(xt[:, :, ts(1, 512)], xt_r[:, :, ts(1, 512)])
        nc.sync.dma_start(wqk[:, :, P : 3 * P], wqk_r[:, :, P : 3 * P])
        nc.sync.dma_start(wqk[:, :, 4 * P : 6 * P], wqk_r[:, :, 4 * P : 6 * P])
        nc.sync.dma_start(xt[:, :, ts(2, 512)], xt_r[:, :, ts(2, 512)])
        nc.sync.dma_start(wo, wo_r)
        nc.sync.dma_start(xt[:, :, ts(3, 512)], xt_r[:, :, ts(3, 512)])

        # replicate v-bias across partitions; set the per-head ones columns
        nc.gpsimd.partition_broadcast(brep, bvrow)
        vnat4 = vnat[:, :, :].rearrange("p a (h e) -> p a h e", e=VG)
        nc.vector.memset(vnat4[:, :, :, HD : HD + 1], 1.0)
        brep3 = brep[:, :].rearrange("p (h e) -> p h e", e=HD)
        # 0/1 lower-triangle mask (keep q >= k): applied by a cheap DVE
        # multiply after the exp
        trimask = persist.tile([P, P], BF16)
        nc.vector.memset(trimask, 1.0)
        nc.gpsimd.affine_select(
            out=trimask, in_=trimask,
            compare_op=mybir.AluOpType.is_ge,
            fill=0.0, base=0, channel_multiplier=-1,
            pattern=[[1, P]],
        )

        def emit_qkv(ob, tt):
            # qkv^T[o, t] = sum_c Wqk^T[c, o] x^T[c, t] + b[o]
            pq = ps512.tile([P, 512], F32, tag="mm")
            for cs in range(CS):
                nc.tensor.matmul(
                    pq,
                    wqk[:, cs, ts(ob, P)],
                    xt[:, cs, ts(tt, 512)],
                    start=(cs == 0),
                    stop=(cs == CS - 1),
                )
            nc.vector.tensor_scalar_add(
                qkvT[:, ob, ts(tt, 512)], pq, bsb[:, ob : ob + 1]
            )

        def emit_vnat(tb):
            # v[t, j] = sum_c x^T[c, t] Wv^T[c, j]  (+ bias via brep)
            pv = ps512.tile([P, 512], F32, tag="mm")
            for cs in range(CS):
                nc.tensor.matmul(
                    pv[:, 0:J],
                    xt[:, cs, ts(tb, P)],
                    wv[:, cs, :],
                    start=(cs == 0),
                    stop=(cs == CS - 1),
                )
            nc.vector.tensor_add(
                out=vnat4[:, tb, :, 0:HD],
                in0=pv[:, 0:J].rearrange("p (h e) -> p h e", e=HD),
                in1=brep3,
            )

        # per-(output, tt) staging: 6 ob units copy into one tile, 1 DMA ships
        # it (coalesced transfer; keeps the HWDGE DMA count low)
        osb_tiles = {}

        def emit_outproj(tt, ob, js_list, okey, copy_eng):
            # part^T[o, t] = sum_{j in js_list} Wo^T[j, o] y^T[j, t]
            po = ps512.tile([P, 512], F32, tag="mm")
            for i, js in enumerate(js_list):
                nc.tensor.matmul(
                    po,
                    wo[:, js, ts(ob, P)],
                    yT[:, js, ts(tt, 512)],
                    start=(i == 0),
                    stop=(i == len(js_list) - 1),
                )
            out_r = outa_r if okey == "a" else outb_r
            if (okey, tt) not in osb_tiles:
                osb_tiles[(okey, tt)] = stage.tile(
                    [P, OUTB, 512], BF16, tag="ld", name=f"osb_{okey}_{tt}",
                    bufs=3,
                )
            osb = osb_tiles[(okey, tt)]
            if copy_eng == "act":
                nc.scalar.copy(osb[:, ob, :], po)
            else:
                nc.vector.tensor_copy(osb[:, ob, :], po)
            if okey == "b" and tt == 3:
                # endgame: ship per-ob so the last DMA is small
                if ob < 3:
                    if ob == 2:
                        nc.sync.dma_start(
                            outb_r[:, 0:3, ts(tt, 512)], osb[:, 0:3, :]
                        )
                else:
                    nc.sync.dma_start(
                        outb_r[:, ob, ts(tt, 512)], osb[:, ob, :]
                    )
            elif ob == OUTB - 1:
                nc.sync.dma_start(out_r[:, :, ts(tt, 512)], osb)

        # ---- phase 1: only what head 0's first half needs (QKV group 0 for
        # q-columns 0-1023, v for k-blocks 0-7); the rest becomes filler
        # inside head 0's ACT-paced stages
        emit_qkv(0, 0)
        emit_vnat(0)
        emit_vnat(1)
        emit_vnat(2)
        emit_vnat(3)
        emit_qkv(3, 0)
        emit_qkv(0, 1)
        for tb in range(4, 8):
            emit_vnat(tb)
        emit_qkv(3, 1)

        # ---- filler work sprinkled into ACT-paced attention stages.
        # (head, hf, kb) -> list of thunks, run after the stage's PV.
        fillers = {}

        def add_filler(key, fn):
            fillers.setdefault(key, []).append(fn)

        # deferred phase-1 tail: QKV group 0 tt 2-3 + v-natural tb 8-15 land
        # inside head 0's early stages (needed from its hf1 half onward)
        for key, fn in (
            ((0, 0, 1), lambda: emit_qkv(0, 2)),
            ((0, 0, 2), lambda: emit_qkv(0, 3)),
            ((0, 0, 3), lambda: emit_vnat(8)),
            ((0, 0, 4), lambda: emit_qkv(3, 2)),
            ((0, 0, 5), lambda: emit_vnat(9)),
            ((0, 0, 6), lambda: emit_qkv(3, 3)),
            ((0, 0, 7), lambda: emit_vnat(10)),
            ((0, 1, 0), lambda: emit_vnat(11)),
            ((0, 1, 1), lambda: emit_vnat(12)),
            ((0, 1, 2), lambda: emit_vnat(13)),
            ((0, 1, 3), lambda: emit_vnat(14)),
            ((0, 1, 4), lambda: emit_vnat(15)),
        ):
            add_filler(key, fn)
        # QKV group 1 during heads 0-1, group 2 during heads 2-3
        for h, ob in ((0, 1), (1, 4), (2, 2), (3, 5)):
            for i, kb in enumerate((2, 5, 9, 13)):
                if h == 0:
                    kb = (6, 8, 10, 12)[i]
                add_filler((h, 1, kb), lambda ob=ob, tt=i: emit_qkv(ob, tt))
        # output projection partial A (heads 0-3, js 0..1): tt 0-1 during
        # head 3 (its yT blocks land as head 3 normalizes), tt 2-3 during
        # head 4; one unit per slot so the PE filler spreads across stages
        oa_slots = {0: [(3, 1, 0), (3, 1, 1), (3, 1, 3), (3, 1, 4),
                        (3, 1, 6), (3, 1, 7)],
                    1: [(3, 1, 8), (3, 1, 10), (3, 1, 11), (3, 1, 12),
                        (3, 1, 14), (3, 1, 15)],
                    2: [(4, 0, 0), (4, 0, 1), (4, 0, 2), (4, 0, 3),
                        (4, 0, 5), (4, 0, 6)],
                    3: [(4, 0, 7), (4, 1, 0), (4, 1, 2), (4, 1, 4),
                        (4, 1, 6), (4, 1, 8)]}
        # output projection remainder B (js 2, heads 4-5) for tt 0-1 during
        # head 5; tt 2-3 go in the endgame drain
        obr_slots = {0: [(5, 1, 0), (5, 1, 1), (5, 1, 2), (5, 1, 3),
                         (5, 1, 4), (5, 1, 5)],
                     1: [(5, 1, 6), (5, 1, 7), (5, 1, 8), (5, 1, 9),
                         (5, 1, 10), (5, 1, 11)]}
        for slots, js_list, okey in ((oa_slots, [0, 1], "a"),
                                     (obr_slots, [2], "b")):
            for tt, keys in slots.items():
                for ob in range(OUTB):
                    add_filler(keys[ob], lambda tt=tt, ob=ob, js=js_list,
                               ok=okey: emit_outproj(tt, ob, js, ok, "dve"))

        # ---- the attention pipeline over all heads.
        # y_ps layout: [128 q-tokens, 2 groups, 512 fp32]; group g holds the
        # four qb tiles (qb%4) at 65-fp32 pitch, so every matmul's accumulate
        # region stays inside one 2KB PSUM bank.
        yps_state = {}

        def emit_pv(ent):
            hl, hf, kb, att, q0, lq = ent
            hp = hl // 2
            if (hl, hf) not in yps_state:
                yps_state[(hl, hf)] = (
                    ps_y.tile([P, 2, 512], F32, tag="y", name="yps"),
                    attn.tile([P, 8], F32, tag="rsb", name="rsb", bufs=2),
                )
            y_ps, rsb = yps_state[(hl, hf)]
            qb_lo = max(kb, 8 * hf)
            for qb in range(qb_lo, 8 * hf + 8):
                ql = qb - 8 * hf           # 0..7 within this half
                g, q = ql // 4, ql % 4
                c0 = qb * P - q0
                # PSUM start_tensor_calc marks the whole 2KB bank pending-
                # zero, so only the bank's FIRST tile may issue start=True;
                # the other three regions' first write consumes the bank-wide
                # pending mark and still gets a fresh (non-accumulating)
                # write.
                nc.tensor.matmul(
                    y_ps[:, g, q * VG : q * VG + VG],
                    att[:, c0 : c0 + P],
                    vnat[:, kb, hl * VG : (hl + 1) * VG],
                    start=(kb == 0 and q == 0),
                    stop=(kb == qb),
                    skip_group_check=True,
                )
            # group g's last tile stops at kb == qb_hi of that group:
            # hf0 groups complete at kb 3 and 7; hf1 at kb 11 and 15.
            if (hf == 0 and kb in (3, 7)) or (hf == 1 and kb in (11, 15)):
                g = (kb - 8 * hf) // 4
                gview = y_ps[:, g, 0 : 4 * VG].rearrange(
                    "p (q e) -> p q e", e=VG
                )
                nc.vector.reciprocal(
                    rsb[:, 4 * g : 4 * g + 4], gview[:, :, HD]
                )
                for q in range(4):
                    tb = 8 * hf + 4 * g + q
                    p0 = (hl % 2) * HD
                    nc.vector.tensor_scalar_mul(
                        ynat[:, tb, hp, p0 : p0 + HD],
                        gview[:, q, 0:HD],
                        rsb[:, 4 * g + q : 4 * g + q + 1],
                    )
                    if hl % 2 == 1:
                        # both heads of the pair done: rebuild y^T via a
                        # DMA xbar transpose (8 tiles, ~112ns engine time)
                        nc.sync.dma_start_transpose(
                            yT[:, hp, tb * P : (tb + 1) * P],
                            ynat[:, tb, hp, :],
                        )

        stages = [
            (hl, hf, kb)
            for hl in range(HL)
            for hf in (0, 1)
            for kb in range(8 if hf == 0 else 16)
        ]

        def emit_scores(sp, kT, qT, kb, q0, lq, j0):
            # scores^T[k, q] into sp[:, j0:j0+lq]
            for j in range(0, lq, 512):
                f = min(512, lq - j)
                nc.tensor.matmul(
                    sp[:, j0 + j : j0 + j + f],
                    kT[:, ts(kb, P)],
                    qT[:, q0 + j : q0 + j + f],
                    start=True,
                    stop=True,
                )

        def emit_mask(att, kb, q0, j0):
            # diagonal block: zero out k > q entries.  att is SBUF bf16, so
            # this can run on the otherwise-idle Pool engine (GPSIMD cannot
            # touch PSUM, but this op never does).
            if kb * P == q0:
                nc.gpsimd.tensor_mul(
                    out=att[:, j0 : j0 + P],
                    in0=att[:, j0 : j0 + P],
                    in1=trimask,
                )

        # tail stages (lq <= 512) are emitted pairwise: both stages' scores
        # share one PSUM tile and a single exp, halving ACT dispatches there
        MERGE = {(0, 4): 5, (0, 6): 7, (1, 12): 13, (1, 14): 15}
        follower_entries = {}
        pending = deque()
        for hl, hf, kb in stages:
            p0 = (hl % 2) * HD
            qT = qkvT[p0 : p0 + HD, hl // 2, :]
            kT = qkvT[p0 : p0 + HD, 3 + hl // 2, :]
            q0 = max(kb * P, hf * 1024)
            lq = (hf + 1) * 1024 - q0
            if (hl, hf, kb) in follower_entries:
                ent = follower_entries.pop((hl, hf, kb))
            elif (hf, kb) in MERGE:
                kb2 = MERGE[(hf, kb)]
                q02 = kb2 * P
                lq2 = (hf + 1) * 1024 - q02
                sp = ps_s.tile([P, 1024], F32, tag="s")
                att = attn.tile([P, 1024], BF16, tag="att", bufs=5)
                emit_scores(sp, kT, qT, kb, q0, lq, 0)
                emit_scores(sp, kT, qT, kb2, q02, lq2, lq)
                nc.scalar.activation(
                    att[:, : lq + lq2], sp[:, : lq + lq2],
                    mybir.ActivationFunctionType.Exp, scale=scale,
                )
                emit_mask(att, kb, q0, 0)
                emit_mask(att, kb2, q02, lq)
                ent = (hl, hf, kb, att[:, 0:lq], q0, lq)
                follower_entries[(hl, hf, kb2)] = (
                    hl, hf, kb2, att[:, lq : lq + lq2], q02, lq2
                )
            else:
                sp = ps_s.tile([P, 1024], F32, tag="s")
                att = attn.tile([P, 1024], BF16, tag="att", bufs=5)
                emit_scores(sp, kT, qT, kb, q0, lq, 0)
                nc.scalar.activation(
                    att[:, :lq], sp[:, :lq],
                    mybir.ActivationFunctionType.Exp, scale=scale,
                )
                emit_mask(att, kb, q0, 0)
                ent = (hl, hf, kb, att[:, 0:lq], q0, lq)
            if len(pending) >= 3:
                emit_pv(pending.popleft())
            pending.append(ent)
            for fn in fillers.get((hl, hf, kb), ()):
                fn()

        # ---- drain: final PVs (whose stops trigger the last normalizes and
        # yT transposes), with the outproj-B remainder interleaved so the PE
        # keeps busy while the DVE/DMA tail completes
        emit_pv(pending.popleft())       # (5,1,13)
        for ob in range(3):
            emit_outproj(2, ob, [2], "b", "act" if ob % 2 else "dve")
        emit_pv(pending.popleft())       # (5,1,14)
        for ob in range(3, OUTB):
            emit_outproj(2, ob, [2], "b", "act" if ob % 2 else "dve")
        emit_pv(pending.popleft())       # (5,1,15): stops qb 12-15
        for ob in range(OUTB):
            emit_outproj(3, ob, [2], "b", "act" if ob % 2 else "dve")


_NC_CACHE = None
LAST_RESULTS = None


def _get_nc():
    global _NC_CACHE
    if _NC_CACHE is None:
        _NC_CACHE = _build_bass()
    return _NC_CACHE


def kernel(x, W_attn, b_attn, W_o, b_o):
    global LAST_RESULTS
    x = np.asarray(x, np.float32)
    W_attn = np.asarray(W_attn, np.float32)
    b_attn = np.asarray(b_attn, np.float32)
    W_o = np.asarray(W_o, np.float32)
    b_o = np.asarray(b_o, np.float32)
    bf = ml_dtypes.bfloat16

    B = x.shape[0]
    in_maps = []
    for core in range(8):
        b, hg = divmod(core, 2)
        sl = slice(hg * J, (hg + 1) * J)
        wq = W_attn[0:C][sl]
        wk = W_attn[C : 2 * C][hg * J : (hg + 1) * J]
        wvl = W_attn[2 * C : 3 * C][hg * J : (hg + 1) * J]
        in_maps.append({
            "xt": np.ascontiguousarray(x[b].T).astype(bf),
            "wqk": np.ascontiguousarray(np.concatenate([wq, wk], 0).T).astype(bf),
            "wv": np.ascontiguousarray(wvl.T).astype(bf),
            "wo": np.ascontiguousarray(W_o[:, sl].T).astype(bf),
            "bqk": np.ascontiguousarray(
                np.concatenate([b_attn[sl], b_attn[C + hg * J : C + (hg + 1) * J]])
            ),
            "bv": np.ascontiguousarray(b_attn[2 * C + hg * J : 2 * C + (hg + 1) * J]),
        })

    nc = _get_nc()
    LAST_RESULTS = bass_utils.run_bass_kernel_spmd(
        nc, in_maps, core_ids=list(range(8)),
        trace=bool(int(os.environ.get("KERNEL_TRACE", "0"))),
    )
    out = np.empty((B, T, C), np.float32)
    for b in range(B):
        acc = None
        for r in (LAST_RESULTS.results[2 * b], LAST_RESULTS.results[2 * b + 1]):
            for key in ("outa", "outb"):
                part = np.asarray(r[key])
                acc = part.astype(np.float32) if acc is None else acc + part
        out[b] = acc.T + b_o
    return out


# revision 19
# speedup vs baseline: 1.0549x; 1.0549x over previous
"""Multi-head causal self-attention (B=4, T=2048, C=768, H=12) on 8 trn2 cores.

Sharding: core c handles batch b = c//2 and head-group hg = c%2 (6 heads each).
Host sums the output-projection partials per batch, transposes back, and adds
b_o. No cross-core collectives.

This revision restructures PV around the cost model's "stationary loads are
free" property: PV runs per (q-block, k-block) 128x128 tile with the att tile
as the stationary operand and vnat (65 cols: 64 v-features + ones) as the
moving operand, cutting PV streaming from 17408 to 8840 columns per head.
The output lands NATURAL [q-token partition, feature], so the softmax
denominator (ones column) sits on the same partition as its token and
normalization is a per-partition DVE reciprocal+multiply -- no Pool
partition-broadcasts at all.  yT for the output projection is rebuilt by
DMA-engine xbar transposes (no PE/PSUM involvement).  Scores/exp/mask and
the QKV/output projections keep the previous structure; output staging
copies are split between DVE and Pool to keep both below the ACT exp load,
which is the end-state bottleneck.
"""

import math
import os
from collections import deque

import numpy as np
import ml_dtypes

import concourse.bass as bass
from concourse import bacc
import concourse.mybir as mybir
import concourse.tile as tile
from concourse import bass_utils
from concourse.bass import ts
from concourse.masks import make_identity

F32 = mybir.dt.float32
BF16 = mybir.dt.bfloat16

P = 128
T = 2048          # sequence length
C = 768           # embed dim
CS = C // P       # 6 contraction chunks
HL = 6            # heads per core
HD = 64           # head dim
J = HL * HD       # 384 local y-feature dim
JS = J // P       # 3
OQK = 2 * J // P  # 6 o-blocks of the local W_qk slice (q rows then k rows)
OUTB = C // P     # 6 output row blocks
TT = T // 512     # 4 column tiles of 512
TB = T // P       # 16 token blocks
VG = HD + 1       # 65: per-head v columns + ones column


F8 = mybir.dt.float8e4


def _build_bass():
    nc = bacc.Bacc("TRN2", target_bir_lowering=False, debug=False)
    xt_d = nc.dram_tensor("xt", [C, T], BF16, kind="ExternalInput").ap()
    xt8_d = nc.dram_tensor("xt8", [C, T], F8, kind="ExternalInput").ap()
    wqk8_d = nc.dram_tensor("wqk8", [C, 2 * J], F8, kind="ExternalInput").ap()
    wv_d = nc.dram_tensor("wv", [C, J], BF16, kind="ExternalInput").ap()
    wo_d = nc.dram_tensor("wo", [J, C], BF16, kind="ExternalInput").ap()
    bqk_d = nc.dram_tensor("bqk", [2 * J], F32, kind="ExternalInput").ap()
    bv_d = nc.dram_tensor("bv", [J], F32, kind="ExternalInput").ap()
    outa_d = nc.dram_tensor("outa", [C, T], BF16, kind="ExternalOutput").ap()
    outb_d = nc.dram_tensor("outb", [C, T], BF16, kind="ExternalOutput").ap()

    with tile.TileContext(nc) as tc, nc.allow_low_precision(
        reason="fp8 QK projection + bf16 matmul pipeline; fp32 PSUM accum"
    ):
        _emit_kernel(tc, xt_d, xt8_d, wqk8_d, wv_d, wo_d, bqk_d, bv_d,
                     outa_d, outb_d)
    nc.compile()
    return nc


# Wqk entries (~uniform ±1/sqrt(768)) sit in fp8e4m3's subnormal range where
# quantization error is 10-20%; pre-scaling by 2^WSHIFT moves them into the
# normal range (~3% error).  q and k then carry a 2^WSHIFT factor each, which
# the exp()'s scale parameter divides back out -- no extra device work.
WSHIFT = 5


def _emit_kernel(tc, xt_d, xt8_d, wqk8_d, wv_d, wo_d, bqk_d, bv_d,
                 outa_d, outb_d):
    nc = tc.nc
    scale = 1.0 / math.sqrt(HD) / (1 << (2 * WSHIFT))

    xt_r = xt_d.rearrange("(cb p) t -> p cb t", p=P)     # [128, 6, 2048]
    # fp8 operands for the QK projection, laid out for DoubleRow matmuls:
    # contraction chunk cs covers channels [cs*256, cs*256+256) as two
    # 128-partition k-tiles stacked on a free axis.
    xt8_r = xt8_d.rearrange("(cs i p) t -> p cs i t", p=P, i=2)
    wqk8_r = wqk8_d.rearrange("(cs i p) o -> p cs i o", p=P, i=2)
    wv_r = wv_d.rearrange("(cb p) j -> p cb j", p=P)     # [128, 6, 384]
    wo_r = wo_d.rearrange("(jb p) o -> p jb o", p=P)     # [128, 3, 768]
    bqk_r = bqk_d.rearrange("(a p) -> p a", p=P)         # [128, 6]
    bv_r = bv_d.rearrange("(p a) -> p a", p=1)           # [1, 384]
    outa_r = outa_d.rearrange("(ob p) t -> p ob t", p=P)  # [128, 6, 2048]
    outb_r = outb_d.rearrange("(ob p) t -> p ob t", p=P)

    with (
        tc.tile_pool(name="persist", bufs=1) as persist,
        tc.tile_pool(name="stage", bufs=2) as stage,
        tc.tile_pool(name="attn", bufs=2) as attn,
        tc.tile_pool(name="ps512", bufs=2, space="PSUM") as ps512,
        tc.tile_pool(name="ps_s", bufs=2, space="PSUM") as ps_s,
        tc.tile_pool(name="ps_y", bufs=1, space="PSUM") as ps_y,
    ):
        xt = persist.tile([P, CS, T], BF16)       # x^T      24KB/partition
        xt8 = persist.tile([P, 3, 2, T], F8)      # x^T fp8  12KB
        wqk8 = persist.tile([P, 3, 2, 2 * J], F8)  # Wqk^T fp8 4.5KB
        wv = persist.tile([P, CS, J], BF16)       # Wv^T    4.5KB
        wo = persist.tile([P, JS, C], BF16)       # Wo^T    4.5KB
        qkvT = persist.tile([P, OQK, T], BF16)    # [q|k]^T  24KB
        vnat = persist.tile([P, TB, HL * VG], BF16)  # v natural 12.2KB
        yT = persist.tile([P, JS, T], BF16)       # y^T      12KB
        ynat = persist.tile([P, TB, JS, P], BF16)  # y natural 12KB
        bsb = persist.tile([P, OQK], F32)
        bvrow = persist.tile([1, J], F32)
        brep = persist.tile([P, J], F32)

        # ---- input loads. HWDGE issues DMAs serially (~625ns each) and the
        # DMA engines run one transfer at a time (internally 16-way), so use
        # FEW large DMAs (>=512B contiguous runs where possible), ordered so
        # the first compute unit's data lands first.
        nc.sync.dma_start(wqk8, wqk8_r)
        nc.sync.dma_start(xt8[:, :, :, ts(0, 512)], xt8_r[:, :, :, ts(0, 512)])
        nc.sync.dma_start(xt[:, :, ts(0, 512)], xt_r[:, :, ts(0, 512)])
        nc.sync.dma_start(wv, wv_r)
        nc.sync.dma_start(bsb, bqk_r)
        nc.sync.dma_start(bvrow, bv_r)
        nc.sync.dma_start(xt8[:, :, :, ts(1, 512)], xt8_r[:, :, :, ts(1, 512)])
        nc.sync.dma_start(xt[:, :, ts(1, 512)], xt_r[:, :, ts(1, 512)])
        nc.sync.dma_start(wo, wo_r)
        nc.sync.dma_start(xt8[:, :, :, ts(2, 512)], xt8_r[:, :, :, ts(2, 512)])
        nc.sync.dma_start(xt[:, :, ts(2, 512)], xt_r[:, :, ts(2, 512)])
        nc.sync.dma_start(xt8[:, :, :, ts(3, 512)], xt8_r[:, :, :, ts(3, 512)])
        nc.sync.dma_start(xt[:, :, ts(3, 512)], xt_r[:, :, ts(3, 512)])

        # replicate v-bias across partitions; set the per-head ones columns
        nc.gpsimd.partition_broadcast(brep, bvrow)
        vnat4 = vnat[:, :, :].rearrange("p a (h e) -> p a h e", e=VG)
        nc.vector.memset(vnat4[:, :, :, HD : HD + 1], 1.0)
        brep3 = brep[:, :].rearrange("p (h e) -> p h e", e=HD)
        # 0/1 lower-triangle mask (keep q >= k): applied by a cheap DVE
        # multiply after the exp
        trimask = persist.tile([P, P], BF16)
        nc.vector.memset(trimask, 1.0)
        nc.gpsimd.affine_select(
            out=trimask, in_=trimask,
            compare_op=mybir.AluOpType.is_ge,
            fill=0.0, base=0, channel_multiplier=-1,
            pattern=[[1, P]],
        )

        def emit_qkv(ob, tt):
            # qk^T[o, t] = sum_c Wqk^T[c, o] x^T[c, t] + b[o], via fp8
            # DoubleRow matmuls: each chunk contracts 256 channels at 0.5
            # cycles/column (4x fewer PE cycles than the bf16 equivalent)
            pq = ps512.tile([P, 512], F32, tag="mm")
            for cs in range(3):
                nc.tensor.matmul(
                    pq,
                    wqk8[:, cs, :, ts(ob, P)],
                    xt8[:, cs, :, ts(tt, 512)],
                    start=(cs == 0),
                    stop=(cs == 2),
                    perf_mode=mybir.MatmulPerfMode.DoubleRow,
                )
            nc.vector.tensor_scalar_add(
                qkvT[:, ob, ts(tt, 512)], pq, bsb[:, ob : ob + 1]
            )

        def emit_vnat(tb):
            # v[t, j] = sum_c x^T[c, t] Wv^T[c, j]  (+ bias via brep)
            pv = ps512.tile([P, 512], F32, tag="mm")
            for cs in range(CS):
                nc.tensor.matmul(
                    pv[:, 0:J],
                    xt[:, cs, ts(tb, P)],
                    wv[:, cs, :],
                    start=(cs == 0),
                    stop=(cs == CS - 1),
                )
            nc.vector.tensor_add(
                out=vnat4[:, tb, :, 0:HD],
                in0=pv[:, 0:J].rearrange("p (h e) -> p h e", e=HD),
                in1=brep3,
            )

        # per-(output, tt) staging: 6 ob units copy into one tile, 1 DMA ships
        # it (coalesced transfer; keeps the HWDGE DMA count low)
        osb_tiles = {}

        def emit_outproj(tt, ob, js_list, okey, copy_eng):
            # part^T[o, t] = sum_{j in js_list} Wo^T[j, o] y^T[j, t]
            po = ps512.tile([P, 512], F32, tag="mm")
            for i, js in enumerate(js_list):
                nc.tensor.matmul(
                    po,
                    wo[:, js, ts(ob, P)],
                    yT[:, js, ts(tt, 512)],
                    start=(i == 0),
                    stop=(i == len(js_list) - 1),
                )
            out_r = outa_r if okey == "a" else outb_r
            if (okey, tt) not in osb_tiles:
                osb_tiles[(okey, tt)] = stage.tile(
                    [P, OUTB, 512], BF16, tag="ld", name=f"osb_{okey}_{tt}",
                    bufs=3,
                )
            osb = osb_tiles[(okey, tt)]
            if copy_eng == "act":
                nc.scalar.copy(osb[:, ob, :], po)
            else:
                nc.vector.tensor_copy(osb[:, ob, :], po)
            if okey == "b" and tt == 3:
                # endgame: ship per-ob so the last DMA is small
                if ob < 3:
                    if ob == 2:
                        nc.sync.dma_start(
                            outb_r[:, 0:3, ts(tt, 512)], osb[:, 0:3, :]
                        )
                else:
                    nc.sync.dma_start(
                        outb_r[:, ob, ts(tt, 512)], osb[:, ob, :]
                    )
            elif ob == OUTB - 1:
                nc.sync.dma_start(out_r[:, :, ts(tt, 512)], osb)

        # ---- phase 1: only what head 0's first half needs (QKV group 0 for
        # q-columns 0-1023, v for k-blocks 0-7); the rest becomes filler
        # inside head 0's ACT-paced stages
        emit_qkv(0, 0)
        emit_vnat(0)
        emit_vnat(1)
        emit_vnat(2)
        emit_vnat(3)
        emit_qkv(3, 0)
        emit_qkv(0, 1)
        for tb in range(4, 8):
            emit_vnat(tb)
        emit_qkv(3, 1)

        # ---- filler work sprinkled into ACT-paced attention stages.
        # (head, hf, kb) -> list of thunks, run after the stage's PV.
        fillers = {}

        def add_filler(key, fn):
            fillers.setdefault(key, []).append(fn)

        # deferred phase-1 tail: QKV group 0 tt 2-3 + v-natural tb 8-15 land
        # inside head 0's early stages (needed from its hf1 half onward)
        for key, fn in (
            ((0, 0, 1), lambda: emit_qkv(0, 2)),
            ((0, 0, 2), lambda: emit_qkv(0, 3)),
            ((0, 0, 3), lambda: emit_vnat(8)),
            ((0, 0, 4), lambda: emit_qkv(3, 2)),
            ((0, 0, 5), lambda: emit_vnat(9)),
            ((0, 0, 6), lambda: emit_qkv(3, 3)),
            ((0, 0, 7), lambda: emit_vnat(10)),
            ((0, 1, 0), lambda: emit_vnat(11)),
            ((0, 1, 1), lambda: emit_vnat(12)),
            ((0, 1, 2), lambda: emit_vnat(13)),
            ((0, 1, 3), lambda: emit_vnat(14)),
            ((0, 1, 4), lambda: emit_vnat(15)),
        ):
            add_filler(key, fn)
        # QKV group 1 during heads 0-1, group 2 during heads 2-3
        for h, ob in ((0, 1), (1, 4), (2, 2), (3, 5)):
            for i, kb in enumerate((2, 5, 9, 13)):
                if h == 0:
                    kb = (6, 8, 10, 12)[i]
                add_filler((h, 1, kb), lambda ob=ob, tt=i: emit_qkv(ob, tt))
        # output projection partial A (heads 0-3, js 0..1): tt 0-1 during
        # head 3 (its yT blocks land as head 3 normalizes), tt 2-3 during
        # head 4; one unit per slot so the PE filler spreads across stages
        oa_slots = {0: [(3, 1, 0), (3, 1, 1), (3, 1, 3), (3, 1, 4),
                        (3, 1, 6), (3, 1, 7)],
                    1: [(3, 1, 8), (3, 1, 10), (3, 1, 11), (3, 1, 12),
                        (3, 1, 14), (3, 1, 15)],
                    2: [(4, 0, 0), (4, 0, 1), (4, 0, 2), (4, 0, 3),
                        (4, 0, 5), (4, 0, 6)],
                    3: [(4, 0, 7), (4, 1, 0), (4, 1, 2), (4, 1, 4),
                        (4, 1, 6), (4, 1, 8)]}
        # output projection remainder B (js 2, heads 4-5) for tt 0-1 during
        # head 5; tt 2-3 go in the endgame drain
        obr_slots = {0: [(5, 1, 0), (5, 1, 1), (5, 1, 2), (5, 1, 3),
                         (5, 1, 4), (5, 1, 5)],
                     1: [(5, 1, 6), (5, 1, 7), (5, 1, 8), (5, 1, 9),
                         (5, 1, 10), (5, 1, 11)]}
        for slots, js_list, okey in ((oa_slots, [0, 1], "a"),
                                     (obr_slots, [2], "b")):
            for tt, keys in slots.items():
                for ob in range(OUTB):
                    add_filler(keys[ob], lambda tt=tt, ob=ob, js=js_list,
                               ok=okey: emit_outproj(tt, ob, js, ok, "dve"))

        # ---- the attention pipeline over all heads.
        # y_ps layout: [128 q-tokens, 2 groups, 512 fp32]; group g holds the
        # four qb tiles (qb%4) at 65-fp32 pitch, so every matmul's accumulate
        # region stays inside one 2KB PSUM bank.
        yps_state = {}

        def emit_pv(ent):
            hl, hf, kb, att, q0, lq = ent
            hp = hl // 2
            p0 = (hl % 2) * HD
            if (hl, hf) not in yps_state:
                yps_state[(hl, hf)] = (
                    ps_y.tile([P, 2, 512], F32, tag="y", name="yps"),
                    attn.tile([P, 8], F32, tag="rsb", name="rsb", bufs=2),
                )
            y_ps, rsb = yps_state[(hl, hf)]
            qb_lo = max(kb, 8 * hf)
            for qb in range(qb_lo, 8 * hf + 8):
                ql = qb - 8 * hf           # 0..7 within this half
                g, q = ql // 4, ql % 4
                c0 = qb * P - q0
                # PSUM start_tensor_calc marks the whole 2KB bank pending-
                # zero, so only the bank's FIRST tile may issue start=True;
                # the other three regions' first write consumes the bank-wide
                # pending mark and still gets a fresh (non-accumulating)
                # write.
                nc.tensor.matmul(
                    y_ps[:, g, q * VG : q * VG + VG],
                    att[:, c0 : c0 + P],
                    vnat[:, kb, hl * VG : (hl + 1) * VG],
                    start=(kb == 0 and q == 0),
                    stop=(kb == qb),
                    skip_group_check=True,
                )
                if kb == qb:
                    # normalize this q-block immediately so its yT transpose
                    # (and the output-projection consumers) see the shortest
                    # possible latency: per-partition reciprocal of the ones-
                    # column denominator + multiply straight out of PSUM.
                    sl = y_ps[:, g, q * VG : q * VG + VG]
                    nc.vector.reciprocal(
                        rsb[:, ql : ql + 1], sl[:, HD : HD + 1]
                    )
                    nc.vector.tensor_scalar_mul(
                        ynat[:, qb, hp, p0 : p0 + HD],
                        sl[:, 0:HD],
                        rsb[:, ql : ql + 1],
                    )
                    if hl % 2 == 1:
                        # both heads of the pair done: rebuild y^T via a
                        # DMA xbar transpose (8 tiles, ~112ns engine time)
                        nc.sync.dma_start_transpose(
                            yT[:, hp, qb * P : (qb + 1) * P],
                            ynat[:, qb, hp, :],
                        )

        stages = [
            (hl, hf, kb)
            for hl in range(HL)
            for hf in (0, 1)
            for kb in range(8 if hf == 0 else 16)
        ]

        def emit_scores(sp, kT, qT, kb, q0, lq, j0):
            # scores^T[k, q] into sp[:, j0:j0+lq]
            for j in range(0, lq, 512):
                f = min(512, lq - j)
                nc.tensor.matmul(
                    sp[:, j0 + j : j0 + j + f],
                    kT[:, ts(kb, P)],
                    qT[:, q0 + j : q0 + j + f],
                    start=True,
                    stop=True,
                )

        def emit_mask(att, kb, q0, j0):
            # diagonal block: zero out k > q entries.  att is SBUF bf16, so
            # this can run on the otherwise-idle Pool engine (GPSIMD cannot
            # touch PSUM, but this op never does).
            if kb * P == q0:
                nc.gpsimd.tensor_mul(
                    out=att[:, j0 : j0 + P],
                    in0=att[:, j0 : j0 + P],
                    in1=trimask,
                )

        # tail stages (lq <= 512) are emitted pairwise: both stages' scores
        # share one PSUM tile and a single exp, halving ACT dispatches there
        MERGE = {(0, 4): 5, (0, 6): 7, (1, 12): 13, (1, 14): 15}
        follower_entries = {}
        pending = deque()
        for hl, hf, kb in stages:
            p0 = (hl % 2) * HD
            qT = qkvT[p0 : p0 + HD, hl // 2, :]
            kT = qkvT[p0 : p0 + HD, 3 + hl // 2, :]
            q0 = max(kb * P, hf * 1024)
            lq = (hf + 1) * 1024 - q0
            if (hl, hf, kb) in follower_entries:
                ent = follower_entries.pop((hl, hf, kb))
            elif (hf, kb) in MERGE:
                kb2 = MERGE[(hf, kb)]
                q02 = kb2 * P
                lq2 = (hf + 1) * 1024 - q02
                sp = ps_s.tile([P, 1024], F32, tag="s")
                att = attn.tile([P, 1024], BF16, tag="att", bufs=5)
                emit_scores(sp, kT, qT, kb, q0, lq, 0)
                emit_scores(sp, kT, qT, kb2, q02, lq2, lq)
                nc.scalar.activation(
                    att[:, : lq + lq2], sp[:, : lq + lq2],
                    mybir.ActivationFunctionType.Exp, scale=scale,
                )
                emit_mask(att, kb, q0, 0)
                emit_mask(att, kb2, q02, lq)
                ent = (hl, hf, kb, att[:, 0:lq], q0, lq)
                follower_entries[(hl, hf, kb2)] = (
                    hl, hf, kb2, att[:, lq : lq + lq2], q02, lq2
                )
            else:
                sp = ps_s.tile([P, 1024], F32, tag="s")
                att = attn.tile([P, 1024], BF16, tag="att", bufs=5)
                emit_scores(sp, kT, qT, kb, q0, lq, 0)
                nc.scalar.activation(
                    att[:, :lq], sp[:, :lq],
                    mybir.ActivationFunctionType.Exp, scale=scale,
                )
                emit_mask(att, kb, q0, 0)
                ent = (hl, hf, kb, att[:, 0:lq], q0, lq)
            if len(pending) >= 2:
                emit_pv(pending.popleft())
            pending.append(ent)
            for fn in fillers.get((hl, hf, kb), ()):
                fn()

        # ---- drain: final PVs (whose stops trigger the last normalizes and
        # yT transposes), with the outproj-B remainder interleaved so the PE
        # keeps busy while the DVE/DMA tail completes
        emit_pv(pending.popleft())       # (5,1,14)
        for ob in range(3):
            emit_outproj(2, ob, [2], "b", "act" if ob % 2 else "dve")
        emit_pv(pending.popleft())       # (5,1,15): stops qb 15
        for ob in range(3, OUTB):
            emit_outproj(2, ob, [2], "b", "act" if ob % 2 else "dve")
        for ob in range(OUTB):
            emit_outproj(3, ob, [2], "b", "act" if ob % 2 else "dve")


_NC_CACHE = None
LAST_RESULTS = None


def _get_nc():
    global _NC_CACHE
    if _NC_CACHE is None:
        _NC_CACHE = _build_bass()
    return _NC_CACHE


def kernel(x, W_attn, b_attn, W_o, b_o):
    global LAST_RESULTS
    x = np.asarray(x, np.float32)
    W_attn = np.asarray(W_attn, np.float32)
    b_attn = np.asarray(b_attn, np.float32)
    W_o = np.asarray(W_o, np.float32)
    b_o = np.asarray(b_o, np.float32)
    bf = ml_dtypes.bfloat16

    f8 = ml_dtypes.float8_e4m3

    B = x.shape[0]
    in_maps = []
    for core in range(8):
        b, hg = divmod(core, 2)
        sl = slice(hg * J, (hg + 1) * J)
        wq = W_attn[0:C][sl]
        wk = W_attn[C : 2 * C][hg * J : (hg + 1) * J]
        wvl = W_attn[2 * C : 3 * C][hg * J : (hg + 1) * J]
        xtb = np.ascontiguousarray(x[b].T)
        wqkT = np.ascontiguousarray(np.concatenate([wq, wk], 0).T)
        wshift = float(1 << WSHIFT)
        in_maps.append({
            "xt": xtb.astype(bf),
            "xt8": xtb.astype(f8),
            "wqk8": (wqkT * wshift).astype(f8),
            "wv": np.ascontiguousarray(wvl.T).astype(bf),
            "wo": np.ascontiguousarray(W_o[:, sl].T).astype(bf),
            "bqk": np.ascontiguousarray(
                np.concatenate([b_attn[sl], b_attn[C + hg * J : C + (hg + 1) * J]])
            ) * wshift,
            "bv": np.ascontiguousarray(b_attn[2 * C + hg * J : 2 * C + (hg + 1) * J]),
        })

    nc = _get_nc()
    LAST_RESULTS = bass_utils.run_bass_kernel_spmd(
        nc, in_maps, core_ids=list(range(8)),
        trace=bool(int(os.environ.get("KERNEL_TRACE", "0"))),
    )
    out = np.empty((B, T, C), np.float32)
    for b in range(B):
        acc = None
        for r in (LAST_RESULTS.results[2 * b], LAST_RESULTS.results[2 * b + 1]):
            for key in ("outa", "outb"):
                part = np.asarray(r[key])
                acc = part.astype(np.float32) if acc is None else acc + part
        out[b] = acc.T + b_o
    return out


# revision 26
# speedup vs baseline: 1.0659x; 1.0105x over previous
"""Multi-head causal self-attention (B=4, T=2048, C=768, H=12) on 8 trn2 cores.

Sharding: core c handles batch b = c//2 and head-group hg = c%2 (6 heads each).
Host sums the output-projection partials per batch, transposes back, and adds
b_o. No cross-core collectives.

This revision restructures PV around the cost model's "stationary loads are
free" property: PV runs per (q-block, k-block) 128x128 tile with the att tile
as the stationary operand and vnat (65 cols: 64 v-features + ones) as the
moving operand, cutting PV streaming from 17408 to 8840 columns per head.
The output lands NATURAL [q-token partition, feature], so the softmax
denominator (ones column) sits on the same partition as its token and
normalization is a per-partition DVE reciprocal+multiply -- no Pool
partition-broadcasts at all.  yT for the output projection is rebuilt by
DMA-engine xbar transposes (no PE/PSUM involvement).  Scores/exp/mask and
the QKV/output projections keep the previous structure; output staging
copies are split between DVE and Pool to keep both below the ACT exp load,
which is the end-state bottleneck.
"""

import math
import os
from collections import deque

import numpy as np
import ml_dtypes

import concourse.bass as bass
from concourse import bacc
import concourse.mybir as mybir
import concourse.tile as tile
from concourse import bass_utils
from concourse.bass import ts
from concourse.masks import make_identity

F32 = mybir.dt.float32
BF16 = mybir.dt.bfloat16

P = 128
T = 2048          # sequence length
C = 768           # embed dim
CS = C // P       # 6 contraction chunks
HL = 6            # heads per core
HD = 64           # head dim
J = HL * HD       # 384 local y-feature dim
JS = J // P       # 3
OQK = 2 * J // P  # 6 o-blocks of the local W_qk slice (q rows then k rows)
OUTB = C // P     # 6 output row blocks
TT = T // 512     # 4 column tiles of 512
TB = T // P       # 16 token blocks
VG = HD + 1       # 65: per-head v columns + ones column


F8 = mybir.dt.float8e4


def _build_bass():
    nc = bacc.Bacc("TRN2", target_bir_lowering=False, debug=False)
    xt_d = nc.dram_tensor("xt", [C, T], BF16, kind="ExternalInput").ap()
    xt8_d = nc.dram_tensor("xt8", [C, T], F8, kind="ExternalInput").ap()
    wqk8_d = nc.dram_tensor("wqk8", [C, 2 * J], F8, kind="ExternalInput").ap()
    wv_d = nc.dram_tensor("wv", [C, J], BF16, kind="ExternalInput").ap()
    wo_d = nc.dram_tensor("wo", [J, C], BF16, kind="ExternalInput").ap()
    bqk_d = nc.dram_tensor("bqk", [2 * J], F32, kind="ExternalInput").ap()
    bv_d = nc.dram_tensor("bv", [J], F32, kind="ExternalInput").ap()
    outa_d = nc.dram_tensor("outa", [C, T], BF16, kind="ExternalOutput").ap()
    outb_d = nc.dram_tensor("outb", [C, T], BF16, kind="ExternalOutput").ap()

    with tile.TileContext(nc) as tc, nc.allow_low_precision(
        reason="fp8 QK projection + bf16 matmul pipeline; fp32 PSUM accum"
    ):
        _emit_kernel(tc, xt_d, xt8_d, wqk8_d, wv_d, wo_d, bqk_d, bv_d,
                     outa_d, outb_d)
    nc.compile()
    return nc


# Wqk entries (~uniform ±1/sqrt(768)) sit in fp8e4m3's subnormal range where
# quantization error is 10-20%; pre-scaling by 2^WSHIFT moves them into the
# normal range (~3% error).  q and k then carry a 2^WSHIFT factor each, which
# the exp()'s scale parameter divides back out -- no extra device work.
WSHIFT = 5


def _emit_kernel(tc, xt_d, xt8_d, wqk8_d, wv_d, wo_d, bqk_d, bv_d,
                 outa_d, outb_d):
    nc = tc.nc
    scale = 1.0 / math.sqrt(HD) / (1 << (2 * WSHIFT))

    xt_r = xt_d.rearrange("(cb p) t -> p cb t", p=P)     # [128, 6, 2048]
    # fp8 operands for the QK projection, laid out for DoubleRow matmuls:
    # contraction chunk cs covers channels [cs*256, cs*256+256) as two
    # 128-partition k-tiles stacked on a free axis.
    xt8_r = xt8_d.rearrange("(cs i p) t -> p cs i t", p=P, i=2)
    wqk8_r = wqk8_d.rearrange("(cs i p) o -> p cs i o", p=P, i=2)
    wv_r = wv_d.rearrange("(cb p) j -> p cb j", p=P)     # [128, 6, 384]
    wo_r = wo_d.rearrange("(jb p) o -> p jb o", p=P)     # [128, 3, 768]
    bqk_r = bqk_d.rearrange("(a p) -> p a", p=P)         # [128, 6]
    bv_r = bv_d.rearrange("(p a) -> p a", p=1)           # [1, 384]
    outa_r = outa_d.rearrange("(ob p) t -> p ob t", p=P)  # [128, 6, 2048]
    outb_r = outb_d.rearrange("(ob p) t -> p ob t", p=P)

    with (
        tc.tile_pool(name="persist", bufs=1) as persist,
        tc.tile_pool(name="stage", bufs=2) as stage,
        tc.tile_pool(name="attn", bufs=2) as attn,
        tc.tile_pool(name="ps512", bufs=2, space="PSUM") as ps512,
        tc.tile_pool(name="ps_s", bufs=2, space="PSUM") as ps_s,
        tc.tile_pool(name="ps_y", bufs=1, space="PSUM") as ps_y,
    ):
        xt = persist.tile([P, CS, T], BF16)       # x^T      24KB/partition
        xt8 = persist.tile([P, 3, 2, T], F8)      # x^T fp8  12KB
        wqk8 = persist.tile([P, 3, 2, 2 * J], F8)  # Wqk^T fp8 4.5KB
        wv = persist.tile([P, CS, J], BF16)       # Wv^T    4.5KB
        wo = persist.tile([P, JS, C], BF16)       # Wo^T    4.5KB
        qkvT = persist.tile([P, OQK, T], BF16)    # [q|k]^T  24KB
        vnat = persist.tile([P, TB, HL * VG], BF16)  # v natural 12.2KB
        yT = persist.tile([P, JS, T], BF16)       # y^T      12KB
        ynat = persist.tile([P, TB, JS, P], BF16)  # y natural 12KB
        bsb = persist.tile([P, OQK], F32)
        bvrow = persist.tile([1, J], F32)
        brep = persist.tile([P, J], F32)

        # ---- input loads. HWDGE issues DMAs serially (~625ns each) and the
        # DMA engines run one transfer at a time (internally 16-way), so use
        # FEW large DMAs (>=512B contiguous runs where possible), ordered so
        # the first compute unit's data lands first.
        nc.sync.dma_start(wqk8, wqk8_r)
        nc.sync.dma_start(xt8[:, :, :, ts(0, 512)], xt8_r[:, :, :, ts(0, 512)])
        nc.sync.dma_start(xt8[:, :, :, ts(1, 512)], xt8_r[:, :, :, ts(1, 512)])
        nc.sync.dma_start(bsb, bqk_r)
        nc.sync.dma_start(xt[:, :, ts(0, 512)], xt_r[:, :, ts(0, 512)])
        nc.sync.dma_start(wv, wv_r)
        nc.sync.dma_start(bvrow, bv_r)
        nc.sync.dma_start(xt8[:, :, :, ts(2, 512)], xt8_r[:, :, :, ts(2, 512)])
        nc.sync.dma_start(xt[:, :, ts(1, 512)], xt_r[:, :, ts(1, 512)])
        nc.sync.dma_start(wo, wo_r)
        nc.sync.dma_start(xt8[:, :, :, ts(3, 512)], xt8_r[:, :, :, ts(3, 512)])
        nc.sync.dma_start(xt[:, :, ts(2, 512)], xt_r[:, :, ts(2, 512)])
        nc.sync.dma_start(xt[:, :, ts(3, 512)], xt_r[:, :, ts(3, 512)])

        # replicate v-bias across partitions; set the per-head ones columns
        nc.gpsimd.partition_broadcast(brep, bvrow)
        vnat4 = vnat[:, :, :].rearrange("p a (h e) -> p a h e", e=VG)
        nc.vector.memset(vnat4[:, :, :, HD : HD + 1], 1.0)
        brep3 = brep[:, :].rearrange("p (h e) -> p h e", e=HD)
        # 0/1 lower-triangle mask (keep q >= k): applied by a cheap DVE
        # multiply after the exp
        trimask = persist.tile([P, P], BF16)
        nc.vector.memset(trimask, 1.0)
        nc.gpsimd.affine_select(
            out=trimask, in_=trimask,
            compare_op=mybir.AluOpType.is_ge,
            fill=0.0, base=0, channel_multiplier=-1,
            pattern=[[1, P]],
        )

        def emit_qkv(ob, tt):
            # qk^T[o, t] = sum_c Wqk^T[c, o] x^T[c, t] + b[o], via fp8
            # DoubleRow matmuls: each chunk contracts 256 channels at 0.5
            # cycles/column (4x fewer PE cycles than the bf16 equivalent)
            pq = ps512.tile([P, 512], F32, tag="mm")
            for cs in range(3):
                nc.tensor.matmul(
                    pq,
                    wqk8[:, cs, :, ts(ob, P)],
                    xt8[:, cs, :, ts(tt, 512)],
                    start=(cs == 0),
                    stop=(cs == 2),
                    perf_mode=mybir.MatmulPerfMode.DoubleRow,
                )
            nc.vector.tensor_scalar_add(
                qkvT[:, ob, ts(tt, 512)], pq, bsb[:, ob : ob + 1]
            )

        def emit_vnat(tb):
            # v[t, j] = sum_c x^T[c, t] Wv^T[c, j]  (+ bias via brep)
            pv = ps512.tile([P, 512], F32, tag="mm")
            for cs in range(CS):
                nc.tensor.matmul(
                    pv[:, 0:J],
                    xt[:, cs, ts(tb, P)],
                    wv[:, cs, :],
                    start=(cs == 0),
                    stop=(cs == CS - 1),
                )
            nc.vector.tensor_add(
                out=vnat4[:, tb, :, 0:HD],
                in0=pv[:, 0:J].rearrange("p (h e) -> p h e", e=HD),
                in1=brep3,
            )

        # per-(output, tt) staging: 6 ob units copy into one tile, 1 DMA ships
        # it (coalesced transfer; keeps the HWDGE DMA count low)
        osb_tiles = {}

        def emit_outproj(tt, ob, js_list, okey, copy_eng):
            # part^T[o, t] = sum_{j in js_list} Wo^T[j, o] y^T[j, t]
            po = ps512.tile([P, 512], F32, tag="mm")
            for i, js in enumerate(js_list):
                nc.tensor.matmul(
                    po,
                    wo[:, js, ts(ob, P)],
                    yT[:, js, ts(tt, 512)],
                    start=(i == 0),
                    stop=(i == len(js_list) - 1),
                )
            out_r = outa_r if okey == "a" else outb_r
            if (okey, tt) not in osb_tiles:
                osb_tiles[(okey, tt)] = stage.tile(
                    [P, OUTB, 512], BF16, tag="ld", name=f"osb_{okey}_{tt}",
                    bufs=3,
                )
            osb = osb_tiles[(okey, tt)]
            if copy_eng == "act":
                nc.scalar.copy(osb[:, ob, :], po)
            else:
                nc.vector.tensor_copy(osb[:, ob, :], po)
            if okey == "b" and tt == 3:
                # endgame: ship per-ob so the last DMA is small
                if ob < 3:
                    if ob == 2:
                        nc.sync.dma_start(
                            outb_r[:, 0:3, ts(tt, 512)], osb[:, 0:3, :]
                        )
                else:
                    nc.sync.dma_start(
                        outb_r[:, ob, ts(tt, 512)], osb[:, ob, :]
                    )
            elif ob == OUTB - 1:
                nc.sync.dma_start(out_r[:, :, ts(tt, 512)], osb)

        # ---- phase 1: the MINIMAL preamble before the first scores -- the
        # cold-clock PE runs ~2x slow for its first ~3us, so every unit here
        # directly delays the first exp.  q/k tiles head 0's first stages
        # read, plus the first two v blocks; everything else lands as filler
        # inside head 0's ACT-paced stages.
        emit_qkv(0, 0)
        emit_qkv(3, 0)
        emit_qkv(0, 1)
        emit_qkv(3, 1)
        emit_vnat(0)
        emit_vnat(1)

        # ---- filler work sprinkled into ACT-paced attention stages.
        # (head, hf, kb) -> list of thunks, run after the stage's PV.
        fillers = {}

        def add_filler(key, fn):
            fillers.setdefault(key, []).append(fn)

        # deferred phase-1 tail: v-natural tb 2-15 and QKV group 0 tt 2-3
        # land inside head 0's stages, each a couple of stages before its
        # first consumer (PV(hf,kb) reads vnat[kb] two stages after stage kb;
        # hf1 scores need q tt2-3 at (0,1,0) and k tt2/tt3 at kb 8/12)
        for key, fn in (
            ((0, 0, 0), lambda: emit_vnat(2)),
            ((0, 0, 1), lambda: emit_vnat(3)),
            ((0, 0, 2), lambda: emit_vnat(4)),
            ((0, 0, 3), lambda: emit_vnat(5)),
            ((0, 0, 4), lambda: emit_vnat(6)),
            ((0, 0, 5), lambda: emit_qkv(0, 2)),
            ((0, 0, 6), lambda: emit_qkv(0, 3)),
            ((0, 0, 7), lambda: emit_vnat(7)),
            ((0, 1, 0), lambda: emit_vnat(8)),
            ((0, 1, 1), lambda: emit_vnat(9)),
            ((0, 1, 2), lambda: emit_vnat(10)),
            ((0, 1, 3), lambda: emit_vnat(11)),
            ((0, 1, 4), lambda: emit_qkv(3, 2)),
            ((0, 1, 5), lambda: emit_vnat(12)),
            ((0, 1, 6), lambda: emit_vnat(13)),
            ((0, 1, 7), lambda: emit_vnat(14)),
            ((0, 1, 8), lambda: emit_vnat(15)),
            ((0, 1, 9), lambda: emit_qkv(3, 3)),
        ):
            add_filler(key, fn)
        # QKV group 1 during heads 0-1 (q blocks in head 1's quiet first
        # half), group 2 during heads 2-3
        for h, hf, ob in ((1, 0, 1), (1, 1, 4), (2, 1, 2), (3, 1, 5)):
            for i, kb in enumerate((2, 5, 9, 13)):
                if hf == 0:
                    kb = (1, 3, 5, 7)[i]
                add_filler((h, hf, kb), lambda ob=ob, tt=i: emit_qkv(ob, tt))
        # output projection partial A (heads 0-3, js 0..1): tt 0-1 during
        # head 3 (its yT blocks land as head 3 normalizes), tt 2-3 during
        # head 4; one unit per slot so the PE filler spreads across stages
        oa_slots = {0: [(3, 1, 0), (3, 1, 1), (3, 1, 3), (3, 1, 4),
                        (3, 1, 6), (3, 1, 7)],
                    1: [(3, 1, 8), (3, 1, 10), (3, 1, 11), (3, 1, 12),
                        (3, 1, 14), (3, 1, 15)],
                    2: [(4, 0, 2), (4, 0, 4), (4, 0, 6), (4, 1, 0),
                        (4, 1, 2), (4, 1, 4)],
                    3: [(4, 1, 6), (4, 1, 8), (4, 1, 10), (4, 1, 12),
                        (4, 1, 14), (5, 0, 1)]}
        # output projection remainder B (js 2, heads 4-5) for tt 0-1 during
        # head 5; tt 2-3 go in the endgame drain
        obr_slots = {0: [(5, 1, 0), (5, 1, 1), (5, 1, 2), (5, 1, 3),
                         (5, 1, 4), (5, 1, 5)],
                     1: [(5, 1, 6), (5, 1, 7), (5, 1, 8), (5, 1, 9),
                         (5, 1, 10), (5, 1, 11)]}
        for slots, js_list, okey in ((oa_slots, [0, 1], "a"),
                                     (obr_slots, [2], "b")):
            for tt, keys in slots.items():
                for ob in range(OUTB):
                    add_filler(keys[ob], lambda tt=tt, ob=ob, js=js_list,
                               ok=okey: emit_outproj(tt, ob, js, ok, "dve"))

        # ---- the attention pipeline over all heads.
        # y_ps layout: [128 q-tokens, 2 groups, 512 fp32]; group g holds the
        # four qb tiles (qb%4) at 65-fp32 pitch, so every matmul's accumulate
        # region stays inside one 2KB PSUM bank.
        yps_state = {}

        def emit_pv(ent):
            hl, hf, kb, att, q0, lq = ent
            hp = hl // 2
            p0 = (hl % 2) * HD
            if (hl, hf) not in yps_state:
                yps_state[(hl, hf)] = (
                    ps_y.tile([P, 2, 512], F32, tag="y", name="yps"),
                    attn.tile([P, 8], F32, tag="rsb", name="rsb", bufs=2),
                )
            y_ps, rsb = yps_state[(hl, hf)]
            qb_lo = max(kb, 8 * hf)
            for qb in range(qb_lo, 8 * hf + 8):
                ql = qb - 8 * hf           # 0..7 within this half
                g, q = ql // 4, ql % 4
                c0 = qb * P - q0
                # PSUM start_tensor_calc marks the whole 2KB bank pending-
                # zero, so only the bank's FIRST tile may issue start=True;
                # the other three regions' first write consumes the bank-wide
                # pending mark and still gets a fresh (non-accumulating)
                # write.
                nc.tensor.matmul(
                    y_ps[:, g, q * VG : q * VG + VG],
                    att[:, c0 : c0 + P],
                    vnat[:, kb, hl * VG : (hl + 1) * VG],
                    start=(kb == 0 and q == 0),
                    stop=(kb == qb),
                    skip_group_check=True,
                )
            # Normalize a PSUM bank only once it is completely written (its
            # last q-block stopped): any DVE read of the bank adds a WAR dep
            # onto later PE matmuls writing the same bank, so reading a bank
            # that still accumulates would cost a ~600ns PE->DVE->PE round
            # trip per stage.  hf0 banks finish at kb 3 and 7; hf1 at 11/15.
            if (hf == 0 and kb in (3, 7)) or (hf == 1 and kb in (11, 15)):
                g = (kb - 8 * hf) // 4
                gview = y_ps[:, g, 0 : 4 * VG].rearrange(
                    "p (q e) -> p q e", e=VG
                )
                nc.vector.reciprocal(
                    rsb[:, 4 * g : 4 * g + 4], gview[:, :, HD]
                )
                for q in range(4):
                    tb = 8 * hf + 4 * g + q
                    nc.vector.tensor_scalar_mul(
                        ynat[:, tb, hp, p0 : p0 + HD],
                        gview[:, q, 0:HD],
                        rsb[:, 4 * g + q : 4 * g + q + 1],
                    )
                    if hl % 2 == 1:
                        # both heads of the pair done: rebuild y^T via a
                        # DMA xbar transpose (8 tiles, ~112ns engine time)
                        nc.sync.dma_start_transpose(
                            yT[:, hp, tb * P : (tb + 1) * P],
                            ynat[:, tb, hp, :],
                        )

        stages = [
            (hl, hf, kb)
            for hl in range(HL)
            for hf in (0, 1)
            for kb in range(8 if hf == 0 else 16)
        ]

        def emit_scores(sp, kT, qT, kb, q0, lq, j0):
            # scores^T[k, q] into sp[:, j0:j0+lq]
            for j in range(0, lq, 512):
                f = min(512, lq - j)
                nc.tensor.matmul(
                    sp[:, j0 + j : j0 + j + f],
                    kT[:, ts(kb, P)],
                    qT[:, q0 + j : q0 + j + f],
                    start=True,
                    stop=True,
                )

        def emit_mask(att, kb, q0, j0):
            # diagonal block: zero out k > q entries.  att is SBUF bf16, so
            # this can run on the otherwise-idle Pool engine (GPSIMD cannot
            # touch PSUM, but this op never does).
            if kb * P == q0:
                nc.gpsimd.tensor_mul(
                    out=att[:, j0 : j0 + P],
                    in0=att[:, j0 : j0 + P],
                    in1=trimask,
                )

        # tail stages (lq <= 512) are emitted pairwise: both stages' scores
        # share one PSUM tile and a single exp, halving ACT dispatches there
        MERGE = {(0, 4): 5, (0, 6): 7, (1, 12): 13, (1, 14): 15}
        follower_entries = {}
        pending = deque()
        for hl, hf, kb in stages:
            p0 = (hl % 2) * HD
            qT = qkvT[p0 : p0 + HD, hl // 2, :]
            kT = qkvT[p0 : p0 + HD, 3 + hl // 2, :]
            q0 = max(kb * P, hf * 1024)
            lq = (hf + 1) * 1024 - q0
            if (hl, hf, kb) in follower_entries:
                ent = follower_entries.pop((hl, hf, kb))
            elif (hf, kb) in MERGE:
                kb2 = MERGE[(hf, kb)]
                q02 = kb2 * P
                lq2 = (hf + 1) * 1024 - q02
                sp = ps_s.tile([P, 1024], F32, tag="s")
                att = attn.tile([P, 1024], BF16, tag="att", bufs=5)
                emit_scores(sp, kT, qT, kb, q0, lq, 0)
                emit_scores(sp, kT, qT, kb2, q02, lq2, lq)
                nc.scalar.activation(
                    att[:, : lq + lq2], sp[:, : lq + lq2],
                    mybir.ActivationFunctionType.Exp, scale=scale,
                )
                emit_mask(att, kb, q0, 0)
                emit_mask(att, kb2, q02, lq)
                ent = (hl, hf, kb, att[:, 0:lq], q0, lq)
                follower_entries[(hl, hf, kb2)] = (
                    hl, hf, kb2, att[:, lq : lq + lq2], q02, lq2
                )
            else:
                sp = ps_s.tile([P, 1024], F32, tag="s")
                att = attn.tile([P, 1024], BF16, tag="att", bufs=5)
                emit_scores(sp, kT, qT, kb, q0, lq, 0)
                nc.scalar.activation(
                    att[:, :lq], sp[:, :lq],
                    mybir.ActivationFunctionType.Exp, scale=scale,
                )
                emit_mask(att, kb, q0, 0)
                ent = (hl, hf, kb, att[:, 0:lq], q0, lq)
            if len(pending) >= 2:
                emit_pv(pending.popleft())
            pending.append(ent)
            for fn in fillers.get((hl, hf, kb), ()):
                fn()

        # ---- drain: final PVs (whose stops trigger the last normalizes and
        # yT transposes), with the outproj-B remainder interleaved so the PE
        # keeps busy while the DVE/DMA tail completes
        emit_pv(pending.popleft())       # (5,1,14)
        for ob in range(3):
            emit_outproj(2, ob, [2], "b", "act" if ob % 2 else "dve")
        emit_pv(pending.popleft())       # (5,1,15): stops qb 15
        for ob in range(3, OUTB):
            emit_outproj(2, ob, [2], "b", "act" if ob % 2 else "dve")
        for ob in range(OUTB):
            emit_outproj(3, ob, [2], "b", "act" if ob % 2 else "dve")


_NC_CACHE = None
LAST_RESULTS = None


def _get_nc():
    global _NC_CACHE
    if _NC_CACHE is None:
        _NC_CACHE = _build_bass()
    return _NC_CACHE


def kernel(x, W_attn, b_attn, W_o, b_o):
    global LAST_RESULTS
    x = np.asarray(x, np.float32)
    W_attn = np.asarray(W_attn, np.float32)
    b_attn = np.asarray(b_attn, np.float32)
    W_o = np.asarray(W_o, np.float32)
    b_o = np.asarray(b_o, np.float32)
    bf = ml_dtypes.bfloat16

    f8 = ml_dtypes.float8_e4m3

    B = x.shape[0]
    in_maps = []
    for core in range(8):
        b, hg = divmod(core, 2)
        sl = slice(hg * J, (hg + 1) * J)
        wq = W_attn[0:C][sl]
        wk = W_attn[C : 2 * C][hg * J : (hg + 1) * J]
        wvl = W_attn[2 * C : 3 * C][hg * J : (hg + 1) * J]
        xtb = np.ascontiguousarray(x[b].T)
        wqkT = np.ascontiguousarray(np.concatenate([wq, wk], 0).T)
        wshift = float(1 << WSHIFT)
        in_maps.append({
            "xt": xtb.astype(bf),
            "xt8": xtb.astype(f8),
            "wqk8": (wqkT * wshift).astype(f8),
            "wv": np.ascontiguousarray(wvl.T).astype(bf),
            "wo": np.ascontiguousarray(W_o[:, sl].T).astype(bf),
            "bqk": np.ascontiguousarray(
                np.concatenate([b_attn[sl], b_attn[C + hg * J : C + (hg + 1) * J]])
            ) * wshift,
            "bv": np.ascontiguousarray(b_attn[2 * C + hg * J : 2 * C + (hg + 1) * J]),
        })

    nc = _get_nc()
    LAST_RESULTS = bass_utils.run_bass_kernel_spmd(
        nc, in_maps, core_ids=list(range(8)),
        trace=bool(int(os.environ.get("KERNEL_TRACE", "0"))),
    )
    out = np.empty((B, T, C), np.float32)
    for b in range(B):
        acc = None
        for r in (LAST_RESULTS.results[2 * b], LAST_RESULTS.results[2 * b + 1]):
            for key in ("outa", "outb"):
                part = np.asarray(r[key])
                acc = part.astype(np.float32) if acc is None else acc + part
        out[b] = acc.T + b_o
    return out
